# revision 21
# baseline (speedup 1.0000x reference)
"""Trainium2 Bass kernel: GQA flash-decoding with paged KV cache (sparse attention).

Problem: B=32 requests, HQ=32 q heads, HKV=8 kv heads, D=128, S=4096 max ctx.
reference = scatter fresh (xk,xv) into kv_buffer at cur_select_index, gather
per-request KV via b_req_tokens_table, masked softmax(q@k^T/sqrt(D)) @ v.

Strategy (request-parallel over 8 cores, no collectives):
 - Host: sort requests by seq_len, snake-assign 4 per core so slot j has a
   similar length on every core.  Ship each core ONLY the valid KV rows
   (plus padding to the per-slot band max) -> ~70% of the dense traffic.
   The fresh token is shipped separately and occupies sequence position 0.
 - Device: for each 128-token chunk: PE-transpose K heads, scoresT[tok,g] =
   K @ qT per kv head (transposed space keeps every matmul's PSUM base
   partition 32-aligned), exp via ACT with the validity mask fused into the
   per-partition bias, then l += ones^T @ p and accT[d, rkg] += V^T @ p
   accumulated in PSUM across chunks.  Final: out = (accT / l)^T.
"""

import os
import sys
from contextlib import ExitStack

import numpy as np

_REPO = os.environ.get("TRN_RL_REPO", "/opt/trn_rl_repo")
if _REPO not in sys.path:
    sys.path.insert(0, _REPO)

import concourse.bass as bass  # noqa: E402
import concourse.tile as tile  # noqa: E402
from concourse import mybir  # noqa: E402
from concourse.bass_utils import run_bass_kernel_spmd  # noqa: E402
from concourse.masks import make_identity  # noqa: E402

B, HQ, HKV, D, S = 32, 32, 8, 128, 4096
G = HQ // HKV  # 4 q heads per kv head
N_CORES = 8
SLOTS = B // N_CORES  # 4 requests per core
ROW = 2 * HKV * D  # 2048 f32 per kv row (8 K heads + 8 V heads)
NEG = np.float32(-1.0e30)
QK_SCALE = float(1.0 / np.sqrt(D))
F32 = mybir.dt.float32
F32R = mybir.dt.float32r  # single-pass tf32-style matmul (f32 is 2-pass hi/lo)
BF16 = mybir.dt.bfloat16


def _legalize_waits(nc):
    """This walrus build accepts at most ONE sync wait per instruction
    ("Too many sync wait commands").  Tile's semaphore assignment emits
    multi-waits; hoist all but the last wait of each instruction onto
    freshly inserted same-engine NOPs placed immediately before it (the
    engine blocks at the NOP instead of at the instruction — equivalent)."""
    counter = 0
    for fn in nc.m.functions:
        for bb in fn.blocks:
            out = []
            for inst in bb.instructions:
                si = inst.sync_info
                waits = list(si.on_wait) if (si and si.on_wait) else []
                if len(waits) > 1:
                    for w in waits[:-1]:
                        nop = mybir.InstNoOp(
                            name=f"WSPLIT-{counter}",
                            engine=inst.engine,
                            ins=[],
                            outs=[],
                            sync_info=mybir.SyncInfo(on_wait=[w], on_update=[]),
                        )
                        counter += 1
                        out.append(nop)
                    si.on_wait = [waits[-1]]
                out.append(inst)
            bb.instructions = out
    return counter


def _plan(req_len):
    """Assign requests to (core, slot) so each slot's band has similar
    lengths across cores; compute per-slot padded chunk counts (shared by
    all cores -> one SPMD program).  req_len = tokens incl the fresh one."""
    order = np.argsort(-req_len, kind="stable")
    assign = np.zeros((N_CORES, SLOTS), dtype=np.int64)
    for j in range(SLOTS):
        band = order[j * N_CORES : (j + 1) * N_CORES]
        if j % 2 == 1:
            band = band[::-1]
        assign[:, j] = band
    T = req_len[assign]
    band_pad = ((T.max(axis=0) + 127) // 128) * 128  # per-slot padded len
    n_chunks = band_pad // 128
    return assign, band_pad.astype(np.int64), n_chunks.astype(np.int64)


def _build_core_inputs(core_reqs, band_pad, kv_buffer, combined, xq,
                       b_seq_len, b_req_tokens_table, cur_select_index):
    """Build one core's input arrays (pure sharding/marshaling in numpy)."""
    row_starts = np.concatenate([[0], np.cumsum(band_pad)])
    p_total = int(row_starts[-1])
    slab = np.zeros((p_total, ROW), dtype=np.float32)
    fresh = np.zeros((SLOTS, ROW), dtype=np.float32)
    qmat = np.zeros((128, D), dtype=np.float32)
    maskb = np.zeros((128, int(np.sum(band_pad) // 128)), dtype=np.float32)

    kv_flat = kv_buffer.reshape(kv_buffer.shape[0], ROW)
    mcol = 0
    for j, req in enumerate(core_reqs):
        L = int(b_seq_len[req])
        idx = b_req_tokens_table[req, :L]
        sel = int(cur_select_index[req])
        pos = np.nonzero(idx == sel)[0]
        fresh_visible = pos.size > 0
        if fresh_visible:
            buf_idx = np.delete(idx, pos)
        else:
            buf_idx = idx
        nbuf = buf_idx.shape[0]
        r0 = int(row_starts[j])
        assert nbuf + 1 <= int(band_pad[j])
        # fast path: contiguous index range -> zero-copy slice
        if nbuf > 0:
            if np.all(np.diff(buf_idx) == 1):
                slab[r0 + 1 : r0 + 1 + nbuf] = kv_flat[buf_idx[0] : buf_idx[0] + nbuf]
            else:
                slab[r0 + 1 : r0 + 1 + nbuf] = kv_flat[buf_idx]
        fresh[j] = combined[req]
        qmat[j * HQ : (j + 1) * HQ] = xq[req]
        # mask bias: token position t (= chunk*128 + partition) valid iff
        # t < 1 + nbuf; position 0 is the fresh token (masked if invisible).
        t_valid = 1 + nbuf
        nch = int(band_pad[j]) // 128
        for lc in range(nch):
            base = lc * 128
            col = np.where(np.arange(base, base + 128) < t_valid, 0.0, NEG)
            if lc == 0 and not fresh_visible:
                col[0] = NEG
            maskb[:, mcol] = col
            mcol += 1
    return {"slab": slab, "fresh": fresh, "q": qmat, "maskb": maskb}


def _build_program(band_pad, n_chunks):
    """Emit the SPMD Bass program (identical for every core)."""
    n_ch_total = int(np.sum(n_chunks))
    p_total = int(np.sum(band_pad))
    row_starts = np.concatenate([[0], np.cumsum(band_pad)]).astype(np.int64)

    nc = bass.Bass()
    slab = nc.declare_dram_parameter("slab", [p_total, ROW], F32, isOutput=False)
    fresh = nc.declare_dram_parameter("fresh", [SLOTS, ROW], F32, isOutput=False)
    q_in = nc.declare_dram_parameter("q", [128, D], F32, isOutput=False)
    maskb_in = nc.declare_dram_parameter("maskb", [128, n_ch_total], F32, isOutput=False)
    out = nc.declare_dram_parameter("out", [128, D], F32, isOutput=True)

    with tile.TileContext(nc) as tc, ExitStack() as ctx:
        const_pool = ctx.enter_context(tc.tile_pool(name="const", bufs=1))
        kv_pool = ctx.enter_context(tc.tile_pool(name="kv", bufs=4))
        kvf_pool = ctx.enter_context(tc.tile_pool(name="kvf", bufs=6))
        kt_pool = ctx.enter_context(tc.tile_pool(name="kt", bufs=3))
        p_pool = ctx.enter_context(tc.tile_pool(name="p", bufs=3))
        fin_pool = ctx.enter_context(tc.tile_pool(name="fin", bufs=1))

        ktp_pool = ctx.enter_context(tc.tile_pool(name="ktp", bufs=2, space="PSUM"))
        sc_pool = ctx.enter_context(tc.tile_pool(name="sc", bufs=2, space="PSUM"))
        acc_pool = ctx.enter_context(tc.tile_pool(name="acc", bufs=1, space="PSUM"))
        l_pool = ctx.enter_context(tc.tile_pool(name="l", bufs=1, space="PSUM"))

        ident = const_pool.tile([128, 128], BF16)
        make_identity(nc, ident[:])
        ident_f = const_pool.tile([128, 128], F32)
        make_identity(nc, ident_f[:])
        ones = const_pool.tile([128, 1], BF16)
        nc.gpsimd.memset(ones[:], 1.0)

        maskb = const_pool.tile([128, n_ch_total], F32)
        nc.sync.dma_start(maskb[:], maskb_in[:])

        # qT[d, rkg]: load q natural then PE-transpose.
        q_nat = const_pool.tile([128, D], F32)
        nc.sync.dma_start(q_nat[:], q_in[:])
        q_ps = ktp_pool.tile([128, 128], F32, tag="ktp")
        nc.tensor.transpose(q_ps[:], q_nat[:], ident_f[:])
        qT = const_pool.tile([128, 128], BF16)
        nc.vector.tensor_copy(qT[:], q_ps[:])

        # PSUM-resident accumulators.  Zero them explicitly and accumulate
        # with start=False matmuls only: start=True clears the whole
        # bank's has_written bits (not just the matmul's own elements),
        # which corrupts a bank shared by several column-range writers,
        # and PSUM data persists across NEFF executions.
        accT = acc_pool.tile([128, 128], F32)  # [d, rkg]
        l_ps = l_pool.tile([128, 1], F32)      # [rkg, 1]
        nc.vector.memset(accT[:], 0.0)
        nc.vector.memset(l_ps[:], 0.0)

        mcol = 0
        for j in range(SLOTS):
            nch = int(n_chunks[j])
            r0 = int(row_starts[j])
            for lc in range(nch):
                first = lc == 0
                last = lc == nch - 1
                # fast HWDGE f32 load, then DVE downcast to bf16 (PE wants
                # 2-byte operands for fast weight load).
                kvf = kvf_pool.tile([128, ROW], F32, tag="kvf")
                if first:
                    # row 0 = fresh token (from xk/xv); slab row r0 is a
                    # zero placeholder and is never read.
                    nc.sync.dma_start(kvf[1:128, :], slab[r0 + 1 : r0 + 128, :])
                    nc.sync.dma_start(kvf[0:1, :], fresh[j : j + 1, :])
                else:
                    rr = r0 + 128 * lc
                    nc.sync.dma_start(kvf[:, :], slab[rr : rr + 128, :])
                kv = kv_pool.tile([128, ROW], BF16, tag="kv")
                nc.vector.tensor_copy(kv[:, : ROW // 2], kvf[:, : ROW // 2])
                nc.vector.tensor_copy(kv[:, ROW // 2 :], kvf[:, ROW // 2 :])

                # K^T for all 8 kv heads: PE transpose -> PSUM -> SBUF
                ktp = ktp_pool.tile([128, HKV * 128], BF16, tag="ktp")
                for k in range(HKV):
                    nc.tensor.transpose(
                        ktp[:, k * 128 : (k + 1) * 128],
                        kv[:, k * D : (k + 1) * D],
                        ident[:],
                    )
                kts = kt_pool.tile([128, HKV * 128], BF16, tag="kt")
                half = HKV * 64
                nc.vector.tensor_copy(kts[:, :half], ktp[:, :half])
                nc.vector.tensor_copy(kts[:, half:], ktp[:, half:])

                # scoresT[tok, (k,g)] = K_head @ qT_pair
                sc = sc_pool.tile([128, HQ], F32, tag="sc")
                for k in range(HKV):
                    nc.tensor.matmul(
                        sc[:, k * G : (k + 1) * G],
                        lhsT=kts[:, k * 128 : (k + 1) * 128],
                        rhs=qT[:, j * HQ + k * G : j * HQ + (k + 1) * G],
                        start=True,
                        stop=True,
                    )

                # p = exp(scoresT * qk_scale + mask_bias[token])
                p = p_pool.tile([128, HQ], BF16, tag="p")
                nc.scalar.activation(
                    p[:],
                    sc[:],
                    mybir.ActivationFunctionType.Exp,
                    bias=maskb[:, mcol : mcol + 1],
                    scale=QK_SCALE,
                )
                mcol += 1

                # l[rkg] += sum_tok p   (rows 32j..32j+32)
                glob_last = j == SLOTS - 1 and last
                nc.tensor.matmul(
                    l_ps[j * HQ : (j + 1) * HQ, 0:1],
                    lhsT=p[:],
                    rhs=ones[:],
                    start=False,
                    stop=glob_last,
                    tile_position=(0, j * HQ),
                    skip_group_check=True,
                )

                # accT[d, rkg] += V_head^T @ p_head
                for k in range(HKV):
                    nc.tensor.matmul(
                        accT[:, j * HQ + k * G : j * HQ + (k + 1) * G],
                        lhsT=kv[:, (HKV + k) * D : (HKV + k + 1) * D],
                        rhs=p[:, k * G : (k + 1) * G],
                        start=False,
                        stop=glob_last and k == HKV - 1,
                        skip_group_check=True,
                    )

        # out[rkg, d] = (accT / l)^T
        acc_sb = fin_pool.tile([128, 128], F32)
        nc.vector.tensor_copy(acc_sb[:], accT[:])
        acc_t = ktp_pool.tile([128, 128], F32, tag="ktp")
        nc.tensor.transpose(acc_t[:], acc_sb[:], ident_f[:])
        l_sb = fin_pool.tile([128, 1], F32)
        nc.vector.tensor_copy(l_sb[:], l_ps[:])
        rl = fin_pool.tile([128, 1], F32)
        nc.vector.reciprocal(rl[:], l_sb[:])
        out_sb = fin_pool.tile([128, 128], F32)
        nc.vector.tensor_scalar_mul(out_sb[:], acc_t[:], rl[:])
        nc.sync.dma_start(out[:], out_sb[:])

    _legalize_waits(nc)
    return nc


def kernel(xq, xk, xv, kv_buffer, cur_select_index, b_req_tokens_table, b_seq_len):
    xq = np.asarray(xq, dtype=np.float32)
    xk = np.asarray(xk, dtype=np.float32)
    xv = np.asarray(xv, dtype=np.float32)
    kv_buffer = np.asarray(kv_buffer, dtype=np.float32)
    cur_select_index = np.asarray(cur_select_index)
    b_req_tokens_table = np.asarray(b_req_tokens_table)
    b_seq_len = np.asarray(b_seq_len)
    assert xq.shape == (B, HQ, D) and kv_buffer.shape == (B * S, 2 * HKV, D)

    # tokens the device processes per request: fresh + buffer rows
    # (buffer rows = seq_len minus the scattered position when visible)
    req_len = np.empty(B, dtype=np.int64)
    for i in range(B):
        L = int(b_seq_len[i])
        idx = b_req_tokens_table[i, :L]
        visible = bool(np.any(idx == int(cur_select_index[i])))
        req_len[i] = L if visible else L + 1

    assign, band_pad, n_chunks = _plan(req_len)
    combined = np.concatenate([xk, xv], axis=1).reshape(B, ROW)

    in_maps = []
    for c in range(N_CORES):
        in_maps.append(
            _build_core_inputs(
                assign[c], band_pad, kv_buffer, combined, xq,
                b_seq_len, b_req_tokens_table, cur_select_index,
            )
        )

    nc = _build_program(band_pad, n_chunks)
    res = run_bass_kernel_spmd(nc, in_maps, core_ids=list(range(N_CORES)))

    out_full = np.zeros((B, HQ, D), dtype=np.float32)
    for c in range(N_CORES):
        core_out = res.results[c]["out"].reshape(SLOTS, HQ, D)
        for j in range(SLOTS):
            out_full[assign[c, j]] = core_out[j]
    return out_full


if __name__ == "__main__":
    import reference

    ins = {k: np.asarray(v) for k, v in reference.setup_inputs().items()}
    got = kernel(**ins)
    exp = np.asarray(reference.reference(**ins))
    err = np.abs(got - exp).max() / (np.abs(exp).max() + 1e-30)
    print("max abs err:", np.abs(got - exp).max(), "rel:", err)


# revision 22
# speedup vs baseline: 1.2030x; 1.2030x over previous
"""Trainium2 Bass kernel: GQA flash-decoding with paged KV cache (sparse attention).

Problem: B=32 requests, HQ=32 q heads, HKV=8 kv heads, D=128, S=4096 max ctx.
reference = scatter fresh (xk,xv) into kv_buffer at cur_select_index, gather
per-request KV via b_req_tokens_table, masked softmax(q@k^T/sqrt(D)) @ v.

Strategy (request-parallel over 8 cores, no collectives):
 - Host: sort requests by seq_len, snake-assign 4 per core so slot j has a
   similar length on every core.  Ship each core ONLY the valid KV rows
   (plus padding to the per-slot band max) -> ~70% of the dense traffic.
   The fresh token is shipped separately and occupies sequence position 0.
 - Device: for each 128-token chunk: PE-transpose K heads, scoresT[tok,g] =
   K @ qT per kv head (transposed space keeps every matmul's PSUM base
   partition 32-aligned), exp via ACT with the validity mask fused into the
   per-partition bias, then l += ones^T @ p and accT[d, rkg] += V^T @ p
   accumulated in PSUM across chunks.  Final: out = (accT / l)^T.
"""

import os
import sys
from contextlib import ExitStack

import numpy as np

_REPO = os.environ.get("TRN_RL_REPO", "/opt/trn_rl_repo")
if _REPO not in sys.path:
    sys.path.insert(0, _REPO)

import concourse.bass as bass  # noqa: E402
import concourse.tile as tile  # noqa: E402
from concourse import mybir  # noqa: E402
from concourse.bass_utils import run_bass_kernel_spmd  # noqa: E402
from concourse.masks import make_identity  # noqa: E402

B, HQ, HKV, D, S = 32, 32, 8, 128, 4096
G = HQ // HKV  # 4 q heads per kv head
N_CORES = 8
SLOTS = B // N_CORES  # 4 requests per core
ROW = 2 * HKV * D  # 2048 f32 per kv row (8 K heads + 8 V heads)
NEG = np.float32(-1.0e30)
QK_SCALE = float(1.0 / np.sqrt(D))
F32 = mybir.dt.float32
F32R = mybir.dt.float32r  # single-pass tf32-style matmul (f32 is 2-pass hi/lo)
BF16 = mybir.dt.bfloat16


def _legalize_waits(nc):
    """This walrus build accepts at most ONE sync wait per instruction
    ("Too many sync wait commands").  Tile's semaphore assignment emits
    multi-waits; hoist all but the last wait of each instruction onto
    freshly inserted same-engine NOPs placed immediately before it (the
    engine blocks at the NOP instead of at the instruction — equivalent)."""
    counter = 0
    for fn in nc.m.functions:
        for bb in fn.blocks:
            out = []
            for inst in bb.instructions:
                si = inst.sync_info
                waits = list(si.on_wait) if (si and si.on_wait) else []
                if len(waits) > 1:
                    for w in waits[:-1]:
                        nop = mybir.InstNoOp(
                            name=f"WSPLIT-{counter}",
                            engine=inst.engine,
                            ins=[],
                            outs=[],
                            sync_info=mybir.SyncInfo(on_wait=[w], on_update=[]),
                        )
                        counter += 1
                        out.append(nop)
                    si.on_wait = [waits[-1]]
                out.append(inst)
            bb.instructions = out
    return counter


def _plan(req_len):
    """Assign requests to (core, slot) so each slot's band has similar
    lengths across cores; compute per-slot padded chunk counts (shared by
    all cores -> one SPMD program).  req_len = tokens incl the fresh one."""
    order = np.argsort(-req_len, kind="stable")
    assign = np.zeros((N_CORES, SLOTS), dtype=np.int64)
    for j in range(SLOTS):
        band = order[j * N_CORES : (j + 1) * N_CORES]
        if j % 2 == 1:
            band = band[::-1]
        assign[:, j] = band
    T = req_len[assign]
    band_pad = ((T.max(axis=0) + 127) // 128) * 128  # per-slot padded len
    n_chunks = band_pad // 128
    return assign, band_pad.astype(np.int64), n_chunks.astype(np.int64)


def _build_core_inputs(core_reqs, band_pad, kv_buffer, combined, xq,
                       b_seq_len, b_req_tokens_table, cur_select_index):
    """Build one core's input arrays (pure sharding/marshaling in numpy)."""
    row_starts = np.concatenate([[0], np.cumsum(band_pad)])
    p_total = int(row_starts[-1])
    slab = np.zeros((p_total, ROW), dtype=np.float32)
    fresh = np.zeros((SLOTS, ROW), dtype=np.float32)
    qmat = np.zeros((128, D), dtype=np.float32)
    maskb = np.zeros((128, int(np.sum(band_pad) // 128)), dtype=np.float32)

    kv_flat = kv_buffer.reshape(kv_buffer.shape[0], ROW)
    mcol = 0
    for j, req in enumerate(core_reqs):
        L = int(b_seq_len[req])
        idx = b_req_tokens_table[req, :L]
        sel = int(cur_select_index[req])
        pos = np.nonzero(idx == sel)[0]
        fresh_visible = pos.size > 0
        if fresh_visible:
            buf_idx = np.delete(idx, pos)
        else:
            buf_idx = idx
        nbuf = buf_idx.shape[0]
        r0 = int(row_starts[j])
        assert nbuf + 1 <= int(band_pad[j])
        # fast path: contiguous index range -> zero-copy slice
        if nbuf > 0:
            if np.all(np.diff(buf_idx) == 1):
                slab[r0 + 1 : r0 + 1 + nbuf] = kv_flat[buf_idx[0] : buf_idx[0] + nbuf]
            else:
                slab[r0 + 1 : r0 + 1 + nbuf] = kv_flat[buf_idx]
        fresh[j] = combined[req]
        qmat[j * HQ : (j + 1) * HQ] = xq[req]
        # mask bias: token position t (= chunk*128 + partition) valid iff
        # t < 1 + nbuf; position 0 is the fresh token (masked if invisible).
        t_valid = 1 + nbuf
        nch = int(band_pad[j]) // 128
        for lc in range(nch):
            base = lc * 128
            col = np.where(np.arange(base, base + 128) < t_valid, 0.0, NEG)
            if lc == 0 and not fresh_visible:
                col[0] = NEG
            maskb[:, mcol] = col
            mcol += 1
    return {"slab": slab, "fresh": fresh, "q": qmat, "maskb": maskb}


def _build_program(band_pad, n_chunks):
    """Emit the SPMD Bass program (identical for every core)."""
    n_ch_total = int(np.sum(n_chunks))
    p_total = int(np.sum(band_pad))
    row_starts = np.concatenate([[0], np.cumsum(band_pad)]).astype(np.int64)

    nc = bass.Bass()
    slab = nc.declare_dram_parameter("slab", [p_total, ROW], F32, isOutput=False)
    fresh = nc.declare_dram_parameter("fresh", [SLOTS, ROW], F32, isOutput=False)
    q_in = nc.declare_dram_parameter("q", [128, D], F32, isOutput=False)
    maskb_in = nc.declare_dram_parameter("maskb", [128, n_ch_total], F32, isOutput=False)
    out = nc.declare_dram_parameter("out", [128, D], F32, isOutput=True)

    with tile.TileContext(nc) as tc, ExitStack() as ctx:
        const_pool = ctx.enter_context(tc.tile_pool(name="const", bufs=1))
        kv_pool = ctx.enter_context(tc.tile_pool(name="kv", bufs=8))
        kt_pool = ctx.enter_context(tc.tile_pool(name="kt", bufs=3))
        p_pool = ctx.enter_context(tc.tile_pool(name="p", bufs=3))
        fin_pool = ctx.enter_context(tc.tile_pool(name="fin", bufs=1))

        ktp_pool = ctx.enter_context(tc.tile_pool(name="ktp", bufs=2, space="PSUM"))
        sc_pool = ctx.enter_context(tc.tile_pool(name="sc", bufs=2, space="PSUM"))
        acc_pool = ctx.enter_context(tc.tile_pool(name="acc", bufs=1, space="PSUM"))
        l_pool = ctx.enter_context(tc.tile_pool(name="l", bufs=1, space="PSUM"))

        ident = const_pool.tile([128, 128], BF16)
        make_identity(nc, ident[:])
        ident_f = const_pool.tile([128, 128], F32)
        make_identity(nc, ident_f[:])
        ones = const_pool.tile([128, 1], BF16)
        nc.gpsimd.memset(ones[:], 1.0)

        maskb = const_pool.tile([128, n_ch_total], F32)
        nc.sync.dma_start(maskb[:], maskb_in[:])

        # qT[d, rkg]: load q natural then PE-transpose.
        q_nat = const_pool.tile([128, D], F32)
        nc.sync.dma_start(q_nat[:], q_in[:])
        q_ps = ktp_pool.tile([128, 128], F32, tag="ktp")
        nc.tensor.transpose(q_ps[:], q_nat[:], ident_f[:])
        qT = const_pool.tile([128, 128], BF16)
        nc.vector.tensor_copy(qT[:], q_ps[:])

        # PSUM-resident accumulators.  Zero them explicitly and accumulate
        # with start=False matmuls only: start=True clears the whole
        # bank's has_written bits (not just the matmul's own elements),
        # which corrupts a bank shared by several column-range writers,
        # and PSUM data persists across NEFF executions.
        accT = acc_pool.tile([128, 128], F32)  # [d, rkg]
        l_ps = l_pool.tile([128, 1], F32)      # [rkg, 1]
        nc.vector.memset(accT[:], 0.0)
        nc.vector.memset(l_ps[:], 0.0)

        mcol = 0
        for j in range(SLOTS):
            nch = int(n_chunks[j])
            r0 = int(row_starts[j])
            for lc in range(nch):
                first = lc == 0
                last = lc == nch - 1
                # f32 -> bf16 cast happens inside the DMA (SWDGE): HBM
                # traffic is unchanged, PE gets 2-byte operands (FWL).
                kv = kv_pool.tile([128, ROW], BF16, tag="kv")
                if first:
                    # row 0 = fresh token (from xk/xv); slab row r0 is a
                    # zero placeholder and is never read.
                    nc.gpsimd.dma_start(kv[1:128, :], slab[r0 + 1 : r0 + 128, :])
                    nc.gpsimd.dma_start(kv[0:1, :], fresh[j : j + 1, :])
                else:
                    rr = r0 + 128 * lc
                    nc.gpsimd.dma_start(kv[:, :], slab[rr : rr + 128, :])

                # K^T for all 8 kv heads: PE transpose -> PSUM -> SBUF
                ktp = ktp_pool.tile([128, HKV * 128], BF16, tag="ktp")
                for k in range(HKV):
                    nc.tensor.transpose(
                        ktp[:, k * 128 : (k + 1) * 128],
                        kv[:, k * D : (k + 1) * D],
                        ident[:],
                    )
                kts = kt_pool.tile([128, HKV * 128], BF16, tag="kt")
                half = HKV * 64
                nc.vector.tensor_copy(kts[:, :half], ktp[:, :half])
                nc.vector.tensor_copy(kts[:, half:], ktp[:, half:])

                # scoresT[tok, (k,g)] = K_head @ qT_pair
                sc = sc_pool.tile([128, HQ], F32, tag="sc")
                for k in range(HKV):
                    nc.tensor.matmul(
                        sc[:, k * G : (k + 1) * G],
                        lhsT=kts[:, k * 128 : (k + 1) * 128],
                        rhs=qT[:, j * HQ + k * G : j * HQ + (k + 1) * G],
                        start=True,
                        stop=True,
                    )

                # p = exp(scoresT * qk_scale + mask_bias[token])
                p = p_pool.tile([128, HQ], BF16, tag="p")
                nc.scalar.activation(
                    p[:],
                    sc[:],
                    mybir.ActivationFunctionType.Exp,
                    bias=maskb[:, mcol : mcol + 1],
                    scale=QK_SCALE,
                )
                mcol += 1

                # l[rkg] += sum_tok p   (rows 32j..32j+32)
                glob_last = j == SLOTS - 1 and last
                nc.tensor.matmul(
                    l_ps[j * HQ : (j + 1) * HQ, 0:1],
                    lhsT=p[:],
                    rhs=ones[:],
                    start=False,
                    stop=glob_last,
                    tile_position=(0, j * HQ),
                    skip_group_check=True,
                )

                # accT[d, rkg] += V_head^T @ p_head
                for k in range(HKV):
                    nc.tensor.matmul(
                        accT[:, j * HQ + k * G : j * HQ + (k + 1) * G],
                        lhsT=kv[:, (HKV + k) * D : (HKV + k + 1) * D],
                        rhs=p[:, k * G : (k + 1) * G],
                        start=False,
                        stop=glob_last and k == HKV - 1,
                        skip_group_check=True,
                    )

        # out[rkg, d] = (accT / l)^T
        acc_sb = fin_pool.tile([128, 128], F32)
        nc.vector.tensor_copy(acc_sb[:], accT[:])
        acc_t = ktp_pool.tile([128, 128], F32, tag="ktp")
        nc.tensor.transpose(acc_t[:], acc_sb[:], ident_f[:])
        l_sb = fin_pool.tile([128, 1], F32)
        nc.vector.tensor_copy(l_sb[:], l_ps[:])
        rl = fin_pool.tile([128, 1], F32)
        nc.vector.reciprocal(rl[:], l_sb[:])
        out_sb = fin_pool.tile([128, 128], F32)
        nc.vector.tensor_scalar_mul(out_sb[:], acc_t[:], rl[:])
        nc.sync.dma_start(out[:], out_sb[:])

    _legalize_waits(nc)
    return nc


def kernel(xq, xk, xv, kv_buffer, cur_select_index, b_req_tokens_table, b_seq_len):
    xq = np.asarray(xq, dtype=np.float32)
    xk = np.asarray(xk, dtype=np.float32)
    xv = np.asarray(xv, dtype=np.float32)
    kv_buffer = np.asarray(kv_buffer, dtype=np.float32)
    cur_select_index = np.asarray(cur_select_index)
    b_req_tokens_table = np.asarray(b_req_tokens_table)
    b_seq_len = np.asarray(b_seq_len)
    assert xq.shape == (B, HQ, D) and kv_buffer.shape == (B * S, 2 * HKV, D)

    # tokens the device processes per request: fresh + buffer rows
    # (buffer rows = seq_len minus the scattered position when visible)
    req_len = np.empty(B, dtype=np.int64)
    for i in range(B):
        L = int(b_seq_len[i])
        idx = b_req_tokens_table[i, :L]
        visible = bool(np.any(idx == int(cur_select_index[i])))
        req_len[i] = L if visible else L + 1

    assign, band_pad, n_chunks = _plan(req_len)
    combined = np.concatenate([xk, xv], axis=1).reshape(B, ROW)

    in_maps = []
    for c in range(N_CORES):
        in_maps.append(
            _build_core_inputs(
                assign[c], band_pad, kv_buffer, combined, xq,
                b_seq_len, b_req_tokens_table, cur_select_index,
            )
        )

    nc = _build_program(band_pad, n_chunks)
    res = run_bass_kernel_spmd(nc, in_maps, core_ids=list(range(N_CORES)))

    out_full = np.zeros((B, HQ, D), dtype=np.float32)
    for c in range(N_CORES):
        core_out = res.results[c]["out"].reshape(SLOTS, HQ, D)
        for j in range(SLOTS):
            out_full[assign[c, j]] = core_out[j]
    return out_full


if __name__ == "__main__":
    import reference

    ins = {k: np.asarray(v) for k, v in reference.setup_inputs().items()}
    got = kernel(**ins)
    exp = np.asarray(reference.reference(**ins))
    err = np.abs(got - exp).max() / (np.abs(exp).max() + 1e-30)
    print("max abs err:", np.abs(got - exp).max(), "rel:", err)
